# revision 3
# baseline (speedup 1.0000x reference)
"""Trainium2 Bass kernel v2 for nn_AttnDecoder (protein conv encoder + GO attention).

Data-parallel over batch: 32 samples -> 4 per core x 8 cores, params replicated.

v2 changes vs baseline:
  - conv1 im2col is prebuilt on the host from the embedded sequence
    (aa_emb[seq] gather is host-side, like the GO-table gather): rhs1[b] is
    [75, 2034] bf16 with row 5k+e = em[e, t+k].  conv1 collapses from 256
    accumulating K=512 matmuls per sample to 16 single K=75 matmuls.
  - the whole attention path runs in bf16 (fp32 matmuls stream at 1/4 rate).
  - x2 is stored bf16; context = narrow bf16 mul+reduce pairs on DVE.
  - conv2 sweeps k-outer (weight-stationary over the 4 n-tiles) so the
    first sample streams against the w2 DMA and LDWEIGHTS amortizes.
  - K=128 dummy-matmul warmup flips the HAM clock gate to 2.4GHz before
    the real work; early DMAs ordered by first PE use; tiny transfers on
    the gpsimd queue.
  - rhs1/x1/x2 double-buffered so consecutive samples pipeline.
"""

import numpy as np
import ml_dtypes

import concourse.bass as bass
import concourse.mybir as mybir
import concourse.tile as tile
from concourse.bass_utils import run_bass_kernel_spmd

# ---- problem constants (must match the reference) ----
B, L = 32, 2048
NCORES = 8
BPC = B // NCORES          # samples per core
AA_VOCAB, AA_EMB = 26, 5
C = 256                    # conv2 out channels
C2 = 512                   # conv1 out channels
S = 8                      # section size
CS = C * S                 # 2048
GO = 256                   # go embedding dim
KS = 15                    # conv kernel size
KC = AA_EMB * KS           # 75 = conv1 contraction
L1 = L - KS + 1            # 2034, conv1 output length
P2 = L1 - KS + 1           # 2020, conv2 output length
NSEC = P2 // S             # 252 sections
HEAD = (P2 % S) // 2       # 2, head trim of the section view
N1_TILES = (0, 512, 1024, 1536)
N2_TILES = (0, 505, 1010, 1515)

F32 = mybir.dt.float32
BF16 = mybir.dt.bfloat16

TRACE = False
LAST_RESULT = None
CONV2_ORDER = "n_outer"    # "n_outer" (baseline) or "k_outer" (weight-stationary)

_NC_CACHE = {}


def _build(conv2_order=CONV2_ORDER):
    nc = bass.Bass()
    rhs1_d = nc.dram_tensor("rhs1", [BPC, KC, L1], BF16, kind="ExternalInput")
    w1t_d = nc.dram_tensor("w1t", [KC, C2], BF16, kind="ExternalInput")
    b1_d = nc.dram_tensor("b1", [C2], F32, kind="ExternalInput")
    w2t_d = nc.dram_tensor("w2t", [KS, C2, C], BF16, kind="ExternalInput")
    b2_d = nc.dram_tensor("b2", [C], F32, kind="ExternalInput")
    goT_d = nc.dram_tensor("goT", [GO, BPC], BF16, kind="ExternalInput")
    attnw_d = nc.dram_tensor("attnw", [GO, CS], BF16, kind="ExternalInput")
    out_d = nc.dram_tensor("out", [BPC, CS], F32, kind="ExternalOutput")

    RELU = mybir.ActivationFunctionType.Relu
    EXP = mybir.ActivationFunctionType.Exp
    AX = mybir.AxisListType.X

    with (
        tile.TileContext(nc) as tc,
        tc.tile_pool(name="singles", bufs=1) as singles,
        tc.tile_pool(name="persamp", bufs=2) as persamp,
        tc.tile_pool(name="rhs1p", bufs=BPC) as rhs1p,
        tc.tile_pool(name="big", bufs=2) as big,
        tc.tile_pool(name="mm", bufs=6, space="PSUM") as mmpool,
        tc.tile_pool(name="spsum", bufs=1, space="PSUM") as spsum,
        tc.tile_pool(name="dram", bufs=1, space="DRAM") as dpool,
    ):
        # ---- resident weights / constants.  DMA issue order tracks when
        # the PE first needs each tensor: conv1 path, attention weights,
        # then conv2 taps (consumed incrementally by the k-outer loop). ----
        w1sb = singles.tile([KC, C2], BF16)
        nc.sync.dma_start(w1sb, w1t_d[:, :])
        rhs1s = []
        for b in range(BPC):
            rhs1 = rhs1p.tile([KC, L1], BF16, tag="rhs1")
            rhs1s.append(rhs1)
        nc.sync.dma_start(rhs1s[0], rhs1_d[0])
        # tiny transfers go through the idle GpSimd DMA path so they don't
        # queue behind the rhs1 bulk (conv1's bias gated the whole pipeline
        # by ~3us when issued on sync after rhs1)
        b1sb = singles.tile([128, 4], F32)
        nc.gpsimd.dma_start(b1sb, b1_d.rearrange("(c p) -> p c", p=128))
        b2sb = singles.tile([128, 2], F32)
        nc.gpsimd.dma_start(b2sb, b2_d.rearrange("(c p) -> p c", p=128))
        gosb = singles.tile([128, 2, BPC], BF16)
        nc.gpsimd.dma_start(gosb, goT_d.rearrange("(c p) b -> p c b", p=128))
        # one tile per tap so conv2's first sweep only waits on the taps it
        # has reached, not the whole 3.9MB transfer
        w2sb = []
        for k in range(KS):
            w2sb_k = singles.tile([128, 4, C], BF16, tag=f"w2_{k}")
            nc.sync.dma_start(
                w2sb_k, w2t_d[k].rearrange("(c p) o -> p c o", p=128)
            )
            w2sb.append(w2sb_k)
        awsb = singles.tile([128, 2, CS], BF16)
        nc.sync.dma_start(awsb, attnw_d.rearrange("(c p) n -> p c n", p=128))
        for b in range(1, BPC):
            nc.sync.dma_start(rhs1s[b], rhs1_d[b])
        onesb = singles.tile([1, 128], F32)
        nc.vector.memset(onesb, 1.0)
        onesw = singles.tile([1, 128], BF16)
        nc.vector.memset(onesw, 1.0)

        # ---- HAM warmup: ~5us of throwaway matmuls while the input DMAs
        # are in flight, so the real work starts at the 2.4GHz clock.  Must
        # be full K=128 matmuls: a K=1 warmup exercises one PE row and the
        # activity monitor never counts it as busy. ----
        wlhs = singles.tile([128, 128], BF16)
        nc.vector.memset(wlhs, 0.0)
        wrhs = singles.tile([128, 512], BF16)
        nc.vector.memset(wrhs, 0.0)
        for _ in range(9):
            wps = mmpool.tile([128, 512], F32, tag="mm512")
            nc.tensor.matmul(wps, wlhs, wrhs, start=True, stop=True)

        vdram = dpool.tile([BPC, CS], BF16)
        vsb = singles.tile([BPC, CS], BF16)

        for b in range(BPC):
            rhs1 = rhs1s[b]

            # ---- conv1 + bias + relu -> x1 [512ch, 2034] bf16 ----
            # bias+relu alternates ScalarE/VectorE: the 16 drains otherwise
            # serialize on ACT and delay conv2's start.
            x1 = big.tile([128, 4, L1], BF16, tag="x1")
            for m in range(4):
                for j, n0 in enumerate(N1_TILES):
                    nn = min(512, L1 - n0)
                    ps = mmpool.tile([128, 512], F32, tag="mm512")
                    nc.tensor.matmul(
                        ps[:, :nn],
                        w1sb[:, 128 * m : 128 * (m + 1)],
                        rhs1[:, n0 : n0 + nn],
                        start=True,
                        stop=True,
                    )
                    if j % 2 == 0:
                        nc.scalar.activation(
                            out=x1[:, m, n0 : n0 + nn],
                            in_=ps[:, :nn],
                            func=RELU,
                            bias=b1sb[:, m : m + 1],
                            scale=1.0,
                        )
                    else:
                        nc.vector.tensor_scalar(
                            out=x1[:, m, n0 : n0 + nn],
                            in0=ps[:, :nn],
                            scalar1=b1sb[:, m : m + 1],
                            scalar2=0.0,
                            op0=mybir.AluOpType.add,
                            op1=mybir.AluOpType.max,
                        )

            if b > 0:
                # ---- per-sample V row in matmul-lhsT layout ----
                vmat = persamp.tile([128, 2, S], BF16, tag="vmat")
                with nc.allow_non_contiguous_dma(reason="per-channel gather of V"):
                    nc.sync.dma_start(
                        vmat, vdram[b].rearrange("(c p s) -> p c s", p=128, s=S)
                    )

            # ---- conv2 + bias + relu -> x2 [256ch, 2020] bf16, with the
            # energy matmuls for channel-half c=m interleaved right after
            # that half of x2 completes (shortens the per-sample tail) ----
            x2 = big.tile([128, 2, P2], BF16, tag="x2")
            eng = spsum.tile([1, NSEC], F32, tag="eng")
            for m in range(2):
                # k-outer, weight-stationary sweep: each tap's weight tile is
                # used for 4 kc x 4 n matmuls as soon as it lands, so the
                # first sample streams against the w2 DMA and later LDWs
                # amortize 4x over the n-tiles.
                pss = []
                for t in range(4):
                    ps2 = mmpool.tile([128, 512], F32, tag="mm512")
                    pss.append(ps2)
                idx = 0
                for k in range(KS):
                    for kc in range(4):
                        for t, n0 in enumerate(N2_TILES):
                            nc.tensor.matmul(
                                pss[t][:, :505],
                                w2sb[k][:, kc, 128 * m : 128 * (m + 1)],
                                x1[:, kc, n0 + k : n0 + k + 505],
                                start=(idx == 0),
                                stop=(idx == 4 * KS - 1),
                            )
                        idx += 1
                for t, n0 in enumerate(N2_TILES):
                    if t % 2 == 0:
                        nc.scalar.activation(
                            out=x2[:, m, n0 : n0 + 505],
                            in_=pss[t][:, :505],
                            func=RELU,
                            bias=b2sb[:, m : m + 1],
                            scale=1.0,
                        )
                    else:
                        nc.vector.tensor_scalar(
                            out=x2[:, m, n0 : n0 + 505],
                            in0=pss[t][:, :505],
                            scalar1=b2sb[:, m : m + 1],
                            scalar2=0.0,
                            op0=mybir.AluOpType.add,
                            op1=mybir.AluOpType.max,
                        )
                if b == 0 and m == 0:
                    # ---- V = go_sel @ attn_w -> [BPC, 2048] bf16, slotted
                    # here (PE warm, conv2 already streaming) instead of at
                    # t=0 where it would gate conv2 on the awsb DMA.  The
                    # DRAM roundtrip yields the per-sample [128, 2, 8]
                    # channel-major layout needed as matmul lhsT. ----
                    for n in range(4):
                        vps = mmpool.tile([128, 512], F32, tag="mm512")
                        for c in range(2):
                            nc.tensor.matmul(
                                vps[:BPC, :],
                                gosb[:, c, :],
                                awsb[:, c, 512 * n : 512 * (n + 1)],
                                start=(c == 0),
                                stop=(c == 1),
                            )
                        nc.scalar.copy(
                            vsb[:, 512 * n : 512 * (n + 1)], vps[:BPC, :]
                        )
                    nc.sync.dma_start(vdram[:, :], vsb[:])
                    vmat = persamp.tile([128, 2, S], BF16, tag="vmat")
                    with nc.allow_non_contiguous_dma(reason="V gather"):
                        nc.sync.dma_start(
                            vmat, vdram[b].rearrange("(c p s) -> p c s", p=128, s=S)
                        )
                else:
                    # ---- energies[n] += sum_{q in half} v_q * enc[q, n];
                    # for b==0 both halves land here (vmat arrives late) ----
                    cs = [m] if b > 0 else [0, 1]
                    for c in cs:
                        for s in range(S):
                            nc.tensor.matmul(
                                eng,
                                vmat[:, c, s : s + 1],
                                x2[:, c, HEAD + NSEC * s : HEAD + NSEC * s + NSEC],
                                start=(s == 0 and c == cs[0]
                                       and (m == 0 or b == 0)),
                                stop=(m == 1 and s == S - 1 and c == cs[-1]),
                            )

            # ---- softmax over the 252 sections (partition 0).  The
            # normalization (1/sum) is deferred to the final ctx scale so
            # the reciprocal sits off the exp->broadcast critical path. ----
            negmax = persamp.tile([1, 1], F32)
            nc.vector.reduce_max(negmax, eng, axis=AX, negate=True)
            expd = persamp.tile([1, NSEC], BF16)
            nc.scalar.activation(out=expd, in_=eng, func=EXP, bias=negmax, scale=1.0)

            # broadcast unnormalized exp over 128 partitions (bf16 K=1 MM);
            # the sum rides a second 1-wide MM so the reciprocal is
            # per-partition and off the exp->attnb critical path
            aps = spsum.tile([128, NSEC], F32, tag="abc")
            nc.tensor.matmul(aps, onesw, expd, start=True, stop=True)
            attnb = persamp.tile([128, NSEC], BF16)
            nc.scalar.copy(attnb, aps)
            ssum = persamp.tile([1, 1], F32)
            nc.vector.reduce_sum(ssum, expd, axis=AX)
            sps = spsum.tile([128, 1], F32, tag="eng")
            nc.tensor.matmul(sps, onesb, ssum, start=True, stop=True)
            rsum = persamp.tile([128, 1], F32)
            nc.vector.reciprocal(rsum, sps)

            # ---- ctx[(c,s)] = (sum_n exp[n] * enc[(c,s), n]) / sum ----
            # (16 narrow mul+reduce pairs; wide broadcast-AP variants lower
            # to ~3x slower DVE code)
            ctx_r = persamp.tile([128, 2, S], F32)
            for c in range(2):
                for s in range(S):
                    tmp = persamp.tile([128, NSEC], BF16, tag="tmp")
                    nc.vector.tensor_mul(
                        tmp,
                        x2[:, c, HEAD + NSEC * s : HEAD + NSEC * s + NSEC],
                        attnb,
                    )
                    nc.vector.reduce_sum(ctx_r[:, c, s : s + 1], tmp, axis=AX)
            ctx_t = persamp.tile([128, 2, S], F32)
            nc.vector.tensor_scalar_mul(ctx_t, ctx_r, rsum)
            with nc.allow_non_contiguous_dma(reason="sectioned layout store"):
                nc.sync.dma_start(
                    out_d[b].rearrange("(c p s) -> p c s", p=128, s=S), ctx_t
                )
    return nc


def _hoist_excess_waits(nc, cap=1):
    """Walrus codegen fits only one sem-wait slot on a Matmult (the LDWEIGHTS
    struct), but Tile attaches one wait per producer processor.  Hoist the
    excess waits onto standalone EventSemaphore instructions inserted just
    before the offender on the same engine queue — queues execute in order,
    so this is semantically identical."""
    import json as _json

    bir = _json.loads(nc.to_json_bytes())
    ctr = [0]

    def fix_block(b):
        insts = b.get("instructions")
        if insts:
            new = []
            for ins in insts:
                si = ins.get("sync_info")
                waits = (si or {}).get("on_wait") or []
                if len(waits) > cap:
                    keep = waits[len(waits) - cap :] if cap else []
                    for w in waits[: len(waits) - cap]:
                        ctr[0] += 1
                        new.append(
                            {
                                "debug": ins.get("debug"),
                                "engine": ins["engine"],
                                "ins": [],
                                "name": f"hoistw-{ctr[0]}",
                                "opcode": "EventSemaphore",
                                "outs": [],
                                "sync_info": {"on_update": [], "on_wait": [w]},
                            }
                        )
                    si["on_wait"] = keep
                new.append(ins)
            b["instructions"] = new
        for sb in b.get("blocks") or []:
            fix_block(sb)

    for fnc in bir["functions"]:
        for b in fnc["blocks"]:
            fix_block(b)
    patched = _json.dumps(bir).encode()
    nc.to_json_bytes = lambda: patched
    return ctr[0]


def get_nc():
    key = CONV2_ORDER
    if key not in _NC_CACHE:
        nc = _build(key)
        n = _hoist_excess_waits(nc)
        print(f"hoisted {n} excess matmul waits", flush=True)
        _NC_CACHE[key] = nc
    return _NC_CACHE[key]


def prep_in_maps(
    input_seq,
    input_go_term,
    aa_emb,
    conv1_w,
    conv1_b,
    conv2_w,
    conv2_b,
    go_table,
    attn_w,
    attn_b,
):
    seq = np.asarray(input_seq).astype(np.int64)
    got = np.asarray(input_go_term).astype(np.int64)
    aa = np.asarray(aa_emb).astype(np.float32)
    w1 = np.asarray(conv1_w).astype(np.float32)
    b1 = np.asarray(conv1_b).astype(np.float32)
    w2 = np.asarray(conv2_w).astype(np.float32)
    b2 = np.asarray(conv2_b).astype(np.float32)
    gt = np.asarray(go_table).astype(np.float32)
    aw = np.asarray(attn_w).astype(np.float32)
    # attn_b shifts all of a sample's energies by one constant -> softmax
    # invariant, so it never reaches the device.

    bf = ml_dtypes.bfloat16

    # conv1 weight as lhsT: row 5k+e -> conv1_w[o, e, k]
    w1t = np.ascontiguousarray(
        w1.transpose(2, 1, 0).reshape(KC, C2)
    ).astype(bf)

    # prebuilt conv1 im2col: em[b, e, t] = aa_emb[seq[b, t], e] (bf16 host
    # gather); rhs1[b, 5k+e, t] = em[b, e, t+k]
    em = aa.astype(bf)[seq]                      # [B, L, 5] bf16 values
    em = np.ascontiguousarray(em.transpose(0, 2, 1))  # [B, 5, L]
    st = em.strides
    rhs1 = np.lib.stride_tricks.as_strided(
        em, shape=(B, KS, AA_EMB, L1), strides=(st[0], st[2], st[1], st[2])
    ).reshape(B, KC, L1)
    rhs1 = np.ascontiguousarray(rhs1)

    w2t = np.ascontiguousarray(w2.transpose(2, 1, 0)).astype(bf)  # [15, 512, 256]

    go_sel = gt[got]  # [B, 256]

    in_maps = []
    for core in range(NCORES):
        sl = slice(core * BPC, (core + 1) * BPC)
        in_maps.append(
            {
                "rhs1": np.ascontiguousarray(rhs1[sl]),
                "w1t": w1t,
                "b1": b1,
                "w2t": w2t,
                "b2": b2,
                "goT": np.ascontiguousarray(go_sel[sl].T).astype(bf),
                "attnw": aw.astype(bf),
            }
        )
    return in_maps


def kernel(**inputs):
    global LAST_RESULT
    nc = get_nc()
    in_maps = prep_in_maps(**inputs)
    res = run_bass_kernel_spmd(
        nc, in_maps, core_ids=list(range(NCORES)), trace=TRACE
    )
    LAST_RESULT = res
    return np.concatenate([r["out"] for r in res.results], axis=0)
